# revision 36
# baseline (speedup 1.0000x reference)
"""Cross-attention Trainium2 Bass kernel (fp8 DoubleRow, software-pipelined).

Computes: out = softmax((x@Wq) @ (ctx@Wk)^T / sqrt(D)) @ (ctx@Wv) + x
for x:[B,N,D]=(4,4096,512), ctx:[B,M,C]=(4,4096,768).

Sharding: 8 cores = (batch b in 0..3) x (query-half h in 0..1). Each core
handles 2048 queries against its batch's full 4096-key context. Pure SPMD,
no collectives.

Host prep: shard, transpose to d-major, cast (xT fp8e4, x natural bf16,
ctxT fp8, weights fp8), pack to [128, ...] partition-major so every
tensor loads in one (or few) large DMAs (HWDGE costs ~625ns per DMA
instruction), and fold the Q projection into the K-side weight
(W_kq = Wk @ Wq^T, computed once in f32): S = q k^T = x (ctx W_kq)^T,
so the kernel's score matmuls consume x^T directly. Output is written
in natural [q, d] layout as bf16 (halves store traffic; ~0.06% extra
rounding on a 2% budget); host upcasts and unpacks the q-tile dim.

Device math, all matmuls fp8e4 DoubleRow (2 k-tiles per instruction,
0.5 cyc/row = the 157 TF/s fp8 peak):
  - projections K'^T (d-major, pre-folded weight) and V (key-major)
    accumulate in PSUM pair tiles, evacuated as single [128,1024]
    copies to fp8 SBUF. PSUM is reachable only from ACT and DVE (the
    hardware forbids GpSimd/Pool), and ACT is saturated by the exp
    stream, so evacuations go to DVE (the first two on ACT, which is
    still idle pre-exp, to start the score stream sooner)
  - attention per 512-query window: per key-tile pair, S^T pair in
    PSUM, one exp -> fp8 P8 (ACT); O accumulates in NATURAL [q,d]
    layout (lhsT = P8 slice stationary, V moving) so the softmax
    normalizer is a per-partition scalar: out = (O * (1/L)[q]) + x
    fuses into ONE scalar_tensor_tensor per q-tile
  - denominator L^T accumulates directly in query-partition layout via
    ~free K=1 DoubleRow matmuls against a ones column; accumulation
    order is rotated (2..15,0,1) so its first mm never waits on
    recip(prev) at a window boundary; per-q-tile reciprocals on DVE
    feed the fused epilogue
  - PE work is balanced across the four score/exp windows: w0 carries
    the K-projections (emitted 4 slots ahead of their first score use
    so a cold DMA never blocks the in-order score stream; the first 2
    key-chunks fold into the pstate warm-up), w1 carries the
    V-projections plus the first O chain trailing V availability, and
    w2/w3 run the remaining O chains as ~2 filler pieces per slot
    through the single spare PSUM bank (a chain's opening mm is never
    placed in the same slot as the previous chain's stt, which would
    stall PE on the bank handoff). The tail runs O(3): qt1/qt2 in a
    borrowed score tile (two banks, no stt gating), qt0/qt3 through
    the O ring, sequenced so only one stt+store trails the last mm.
PSUM: score-pair ring 3 x [128,2,512] (6 banks) + O chain (1 bank) +
L accumulator (1 bank) = exactly 8 banks.
"""
import sys


def _ensure_concourse():
    try:
        import concourse  # noqa: F401
    except ImportError:
        for p in ("/opt/trn_rl_repo", "/root/.axon_site/_ro/trn_rl_repo"):
            if p not in sys.path:
                sys.path.insert(0, p)


_ensure_concourse()

import numpy as np
import ml_dtypes

import concourse.bacc as bacc
import concourse.tile as tile
from concourse import mybir
from concourse.bass_utils import run_bass_kernel_spmd

F32 = mybir.dt.float32
BF16 = mybir.dt.bfloat16
F8 = mybir.dt.float8e4
NP_F8 = ml_dtypes.float8_e4m3
NP_BF16 = ml_dtypes.bfloat16
DR = mybir.MatmulPerfMode.DoubleRow
ALU = mybir.AluOpType

DIM = 512
CTX = 768
B, N, M = 4, 4096, 4096
NCORES = 8
QCH = 512
SCALE = float(DIM) ** -0.5

N_DT = DIM // 128   # 4 d tiles
N_CT = CTX // 128   # 6 c tiles


def build_nc(n_q, n_keys):
    """Per-core SPMD program: n_q queries x n_keys context rows."""
    assert n_q % QCH == 0 and n_keys % 1024 == 0
    n_qch = n_q // QCH        # query chunks (4)
    n_kc = n_keys // 512      # key chunks (8)
    n_ktp = n_keys // 256     # key-tile pairs (16)
    n_qt = n_q // 128         # query tiles (16)

    nc = bacc.Bacc(None, target_bir_lowering=False)

    x16_d = nc.dram_tensor("x16", [128, n_qt, DIM], BF16, kind="ExternalInput")
    xT8_d = nc.dram_tensor("xT8", [128, N_DT, n_q], F8, kind="ExternalInput")
    ctx8_d = nc.dram_tensor("ctxT8", [128, N_CT, n_keys], F8, kind="ExternalInput")
    # wk8 holds the folded W_kq = Wk @ Wq^T (host weight prep), so the
    # score matmuls consume xT8 directly and the Q projection disappears
    wk8_d = nc.dram_tensor("wk8", [128, N_CT, DIM], F8, kind="ExternalInput")
    wv8_d = nc.dram_tensor("wv8", [128, N_CT, DIM], F8, kind="ExternalInput")
    out_d = nc.dram_tensor("out", [128, n_qt, DIM], BF16, kind="ExternalOutput")

    ones8_d = nc.inline_tensor(np.ones((128, 2), NP_F8), "ones8")

    with tile.TileContext(nc) as tc:
        with (
            tc.tile_pool(name="const", bufs=1) as const,
            tc.tile_pool(name="res", bufs=1) as res,
            tc.tile_pool(name="p8", bufs=3) as p8_pool,
            tc.tile_pool(name="fin", bufs=3) as fin,
            tc.tile_pool(name="sc", bufs=3, space="PSUM") as sc,
            tc.tile_pool(name="opool", bufs=1, space="PSUM") as opool,
            tc.tile_pool(name="lpool", bufs=1, space="PSUM") as lpool,
        ):
            ones8 = const.tile([128, 2, 1], F8)
            wk8 = res.tile([128, N_CT, DIM], F8)
            wv8 = res.tile([128, N_CT, DIM], F8)
            XT8 = res.tile([128, N_DT, n_q], F8)
            X16 = res.tile([128, n_qt, DIM], BF16)
            CT8 = res.tile([128, N_CT, n_keys], F8)
            KT8 = res.tile([128, N_DT, n_keys], F8)
            V8 = res.tile([128, n_keys // 128, DIM], F8)

            # ---- PE clock warm-up: dummy matmuls on memset data keep the
            # tensor engine "continuously executing" through the DMA ramp so
            # real work starts at full pstate. K-projections for the first
            # two key chunks are woven in as soon as their DMAs land so the
            # score/exp stream starts ~3us earlier (they pay the mid-pstate
            # rate but retire real work). ----
            warm = const.tile([128, 512], BF16)
            nc.gpsimd.memset(warm, 1.0)
            wps = sc.tile([128, 2, QCH], F32, tag="sc", name="warmps")

            def warm_mm(n):
                for w in range(n):
                    nc.tensor.matmul(wps[0:16, 0, :], lhsT=warm[:, 0:16],
                                     rhs=warm[:, :], start=True, stop=True)

            # ---- input DMAs: few and large; ordered by first use.
            # Window 0 runs K-projections only, so ctx-kc0 + wk8 lead (wk8
            # split so the first kproj's half arrives sooner); wv8 is not
            # needed until late window 0, X16 not until the first stt. ----
            nc.sync.dma_start(out=CT8[:, :, 0:512], in_=ctx8_d[:, :, 0:512])
            nc.sync.dma_start(out=wk8[:, :, 0:256], in_=wk8_d[:, :, 0:256])
            nc.sync.dma_start(out=wk8[:, :, 256:512],
                              in_=wk8_d[:, :, 256:512])
            nc.sync.dma_start(out=CT8[:, :, 512:1024],
                              in_=ctx8_d[:, :, 512:1024])
            nc.sync.dma_start(out=XT8[:, :, 0:QCH], in_=xT8_d[:, :, 0:QCH])
            nc.sync.dma_start(out=ones8, in_=ones8_d[:])
            nc.sync.dma_start(out=CT8[:, :, 1024:1536],
                              in_=ctx8_d[:, :, 1024:1536])
            nc.sync.dma_start(out=CT8[:, :, 1536:2048],
                              in_=ctx8_d[:, :, 1536:2048])
            nc.sync.dma_start(out=CT8[:, :, 2048:2560],
                              in_=ctx8_d[:, :, 2048:2560])
            nc.sync.dma_start(out=wv8, in_=wv8_d[:])
            nc.sync.dma_start(out=CT8[:, :, 2560:3072],
                              in_=ctx8_d[:, :, 2560:3072])
            nc.sync.dma_start(out=CT8[:, :, 3072:3584],
                              in_=ctx8_d[:, :, 3072:3584])
            nc.sync.dma_start(out=CT8[:, :, 3584:4096],
                              in_=ctx8_d[:, :, 3584:4096])
            nc.sync.dma_start(out=XT8[:, :, QCH:n_q], in_=xT8_d[:, :, QCH:n_q])
            nc.sync.dma_start(out=X16, in_=x16_d[:])

            # ---- projection pieces (fp8 DoubleRow; pair-tile psum) ----
            evac_ctr = [0]

            def evac(dst, ps, eng=None):
                # PSUM is only reachable from ACT and DVE (GpSimd/Pool is
                # forbidden by the hardware). ACT is saturated by the exp
                # stream, so evacuations default to DVE; the first K-proj
                # evacs override to ACT (idle until the first exp) to get
                # the score stream started sooner.
                if eng is None or eng == "pool":
                    eng = "dve"
                if eng == "act":
                    nc.scalar.copy(out=dst, in_=ps)
                else:
                    nc.vector.tensor_copy(out=dst, in_=ps)

            def vproj_half(kc, half):
                kt0 = kc * 4 + half * 2
                ps = sc.tile([128, 2, DIM], F32, tag="sc",
                             name=f"psv{kc}_{half}")
                for j in (0, 1):
                    kt = kt0 + j
                    for t in (0, 1, 2):
                        nc.tensor.matmul(
                            ps[:, j, :],
                            lhsT=CT8[:, 2 * t:2 * t + 2,
                                     kt * 128:(kt + 1) * 128],
                            rhs=wv8[:, 2 * t:2 * t + 2, :],
                            start=(t == 0), stop=(t == 2), perf_mode=DR)
                evac(V8[:, kt0:kt0 + 2, :], ps)

            def kproj_half(kc, dtp, eng=None):
                ksl = slice(kc * 512, (kc + 1) * 512)
                ps = sc.tile([128, 2, 512], F32, tag="sc",
                             name=f"psk{kc}_{dtp}")
                for j in (0, 1):
                    dt = 2 * dtp + j
                    for t in (0, 1, 2):
                        nc.tensor.matmul(
                            ps[:, j, :],
                            lhsT=wk8[:, 2 * t:2 * t + 2,
                                     dt * 128:(dt + 1) * 128],
                            rhs=CT8[:, 2 * t:2 * t + 2, ksl],
                            start=(t == 0), stop=(t == 2), perf_mode=DR)
                evac(KT8[:, 2 * dtp:2 * dtp + 2, ksl], ps, eng=eng)

            # ---- attention pieces ----
            st = {}   # qc -> tiles

            def attn_begin(qc):
                st[qc] = {
                    "P8t": p8_pool.tile([128, n_ktp, 2, QCH], F8, tag="p8",
                                        name=f"p8_{qc}"),
                    "l_n": lpool.tile([128, QCH], F32, tag="l",
                                      name=f"ln{qc}"),
                    "srcmap": {},
                }

            def emit_s_exp(qc, ktp):
                d = st[qc]
                qsl = slice(qc * QCH, (qc + 1) * QCH)
                s_t = sc.tile([128, 2, QCH], F32, tag="sc",
                              name=f"s{qc}_{ktp}")
                for j in (0, 1):
                    kt = 2 * ktp + j
                    for dtp in (0, 1):
                        nc.tensor.matmul(
                            s_t[:, j, :],
                            lhsT=KT8[:, 2 * dtp:2 * dtp + 2,
                                     kt * 128:(kt + 1) * 128],
                            rhs=XT8[:, 2 * dtp:2 * dtp + 2, qsl],
                            start=(dtp == 0), stop=(dtp == 1), perf_mode=DR)
                nc.scalar.activation(
                    out=d["P8t"][:, ktp, :, :], in_=s_t[:, :, :],
                    func=mybir.ActivationFunctionType.Exp, scale=SCALE)

            def emit_l(qc, ktp, start, stop):
                # ~free L tinies: L^T[q] accumulates via K=1 DoubleRow
                # matmuls against a ones column. Accumulation order is
                # rotated (2..15,0,1) so the chain's first mm does not sit
                # right at a window boundary waiting on recip(prev).
                d = st[qc]
                for qt in range(4):
                    nc.tensor.matmul(
                        d["l_n"][:, qt * 128:qt * 128 + 1],
                        lhsT=d["P8t"][:, ktp, :, qt * 128:(qt + 1) * 128],
                        rhs=ones8,
                        start=start, stop=stop,
                        perf_mode=DR)

            def emit_l_rot(qc, t):
                if t >= 2:
                    emit_l(qc, t, start=(t == 2), stop=False)

            def emit_l_wrap(qc):
                emit_l(qc, 0, start=False, stop=False)
                emit_l(qc, 1, start=False, stop=True)

            def o_mm(qc, ktp, qt, dst, start, stop):
                # natural-layout O: lhsT = P8 slice (stationary), V moving
                nc.tensor.matmul(
                    dst,
                    lhsT=st[qc]["P8t"][:, ktp, :, qt * 128:(qt + 1) * 128],
                    rhs=V8[:, 2 * ktp:2 * ktp + 2, :],
                    start=start, stop=stop, perf_mode=DR)

            def recip_scalars(qc):
                d = st[qc]
                d["rTs"] = fin.tile([128, 4], F32, tag="rTs",
                                    name=f"rTs{qc}")
                for qt in range(4):
                    nc.vector.reciprocal(
                        out=d["rTs"][:, qt:qt + 1],
                        in_=d["l_n"][:, qt * 128:qt * 128 + 1])
                d["ob"] = fin.tile([128, 4, DIM], BF16, tag="ob",
                                   name=f"obs{qc}")

            def chain_alloc(qc, qt, from_sc=False):
                # accumulation bank for one O chain (qc, qt)
                if from_sc:
                    t_ = sc.tile([128, 2, QCH], F32, tag="sc",
                                 name=f"ob{qc}_{qt}")
                    st[qc][f"o_{qt}"] = t_
                    st[qc]["srcmap"][qt] = t_[:, 0, :]
                    st[qc][f"o_{qt + 1}"] = t_
                    st[qc]["srcmap"][qt + 1] = t_[:, 1, :]
                else:
                    t_ = opool.tile([128, 1, QCH], F32, tag="o",
                                    name=f"ob{qc}_{qt}")
                    st[qc][f"o_{qt}"] = t_
                    st[qc]["srcmap"][qt] = t_[:, 0, :]

            def chain_mms(qc, qt, ktps, bank=0, first=False, last=False):
                d = st[qc]
                for i, ktp in enumerate(ktps):
                    o_mm(qc, ktp, qt, d[f"o_{qt}"][:, bank, :],
                         start=(first and i == 0),
                         stop=(last and i == len(ktps) - 1))

            def stt(qc, qt):
                # out = O[qt] * (1/L)[q] + x   — one fused DVE op
                d = st[qc]
                nc.vector.scalar_tensor_tensor(
                    out=d["ob"][:, qt, :], in0=d["srcmap"][qt],
                    scalar=d["rTs"][:, qt:qt + 1],
                    in1=X16[:, qc * 4 + qt, :],
                    op0=ALU.mult, op1=ALU.add)

            def store(qc, qts):
                d = st[qc]
                g0 = qc * 4 + qts[0]
                nc.sync.dma_start(
                    out=out_d[:, g0:g0 + len(qts), :],
                    in_=d["ob"][:, qts[0]:qts[0] + len(qts), :])

            # ---- schedule ----
            # Four score/exp windows (one per query chunk). PE work is
            # balanced so every window carries ~equal matmul load beside
            # its 16-score/exp stream:
            #   w0: K-projections (just-in-time for the scores)
            #   w1: V-projections + O(0,qt0) chain trailing V availability
            #   w2: O chains (0,1..3),(1,0..2) through the single O bank
            #   w3: O chains (1,3),(2,0..3); tail: (3,*) via borrowed
            #       score slots (2 banks -> 2 gating-free chains)
            # window 0: scores + K-projections. All warm-ups run first (PE
            # SEQ is in-order; they fill the initial DMA wait and finish the
            # pstate ramp). kc0/kc1 projections lead with ACT/DVE evacs (ACT
            # is idle until the first exp); later kprojs are emitted 2 slots
            # ahead of their first score use so they never block the score
            # stream on a cold DMA. Slots 14/15 pre-issue the first V-proj
            # halves to carry PE across the window boundary.
            warm_mm(9)
            attn_begin(0)
            kproj_half(0, 0, eng="act"); kproj_half(0, 1, eng="dve")
            kproj_half(1, 0, eng="act"); kproj_half(1, 1, eng="dve")
            for t in range(n_ktp):
                emit_s_exp(0, t)
                emit_l_rot(0, t)
                h = t + 4
                if 4 <= h < 16:
                    kproj_half(h // 2, h % 2)
                elif h >= 16:
                    vproj_half((h - 16) // 2, (h - 16) % 2)
            emit_l_wrap(0)

            # window 1: scores + V-projections + O(0,qt0) trailing chain
            attn_begin(1)
            chain_alloc(0, 0)
            for t in range(n_ktp):
                emit_s_exp(1, t)
                emit_l_rot(1, t)
                if t == 1:
                    recip_scalars(0)
                h = t + 4
                if h < 16:
                    vproj_half(h // 2, h % 2)
                if t >= 2:
                    chain_mms(0, 0, [t - 2], first=(t == 2))
            emit_l_wrap(1)

            # windows 2..3: chain pipeline as fillers through the stream
            def pipeline_pieces(win):
                """Filler (kind, closure) list for window `win` (2 or 3)."""
                ps = []
                if win == 2:
                    ps.append(("x", lambda: (chain_mms(0, 0, [14, 15],
                                                       last=True),
                                             recip_scalars(1))))
                    ps.append(("x", lambda: stt(0, 0)))
                    chains = [(0, 1), (0, 2), (0, 3), (1, 0), (1, 1), (1, 2)]
                else:
                    # recip(2) first: L(2) accumulation completed with w2
                    ps.append(("x", lambda: recip_scalars(2)))
                    chains = [(1, 3), (2, 0), (2, 1), (2, 2), (2, 3)]
                for qc_, qt_ in chains:
                    def mk(qc_, qt_):
                        def begin():
                            chain_alloc(qc_, qt_)
                            chain_mms(qc_, qt_, range(0, 6), first=True)
                        def mid():
                            chain_mms(qc_, qt_, range(6, 11))
                        def end():
                            chain_mms(qc_, qt_, range(11, 16), last=True)
                        def fin_():
                            stt(qc_, qt_)
                            if qt_ == 3:
                                store(qc_, (0, 1, 2, 3))
                        return [("begin", begin), ("mid", mid),
                                ("end", end), ("fin", fin_)]
                    ps.extend(mk(qc_, qt_))
                return ps

            for win in (2, 3):
                attn_begin(win)
                pieces = pipeline_pieces(win)
                pc = 0
                for t in range(n_ktp):
                    emit_s_exp(win, t)
                    emit_l_rot(win, t)
                    # ~2 pieces per slot keeps the O-bank pipeline moving,
                    # but never start a chain in the same slot as the
                    # previous chain's stt: the begin-mm would stall PE on
                    # the stt; deferring it puts score mms in between.
                    take = 2 if pc < len(pieces) - 1 else 1
                    took_fin = False
                    for _ in range(take):
                        if pc >= len(pieces):
                            break
                        kind = pieces[pc][0]
                        if took_fin and kind == "begin":
                            break
                        pieces[pc][1]()
                        took_fin = (kind == "fin")
                        pc += 1
                emit_l_wrap(win)
                for _, p_ in pieces[pc:]:
                    p_()
                if win == 2:
                    st.pop(0)

            # tail: O(3) chains. qt1/qt2 share a borrowed score tile
            # (independent banks, no stt gating); qt0/qt3 use the O ring.
            # Sequenced so only one stt+store remains after PE's last mm.
            recip_scalars(3)
            chain_alloc(3, 0)
            chain_mms(3, 0, range(0, 16), first=True, last=True)
            chain_alloc(3, 1, from_sc=True)   # also claims qt2 (bank 1)
            chain_mms(3, 1, range(0, 16), bank=0, first=True, last=True)
            stt(3, 0)
            chain_mms(3, 2, range(0, 8), bank=1, first=True)
            stt(3, 1)
            store(3, (0, 1))
            chain_mms(3, 2, range(8, 16), bank=1, last=True)
            chain_alloc(3, 3)
            chain_mms(3, 3, range(0, 8), first=True)
            stt(3, 2)
            store(3, (2,))
            chain_mms(3, 3, range(8, 16), last=True)
            stt(3, 3)
            store(3, (3,))
            st.pop(1); st.pop(2); st.pop(3)

    nc.finalize()
    return nc


SHARD_SHAPE = (N // 2, M)   # (n_q, n_keys) per core

_NC_CACHE = {}


def _get_nc(n_q, n_keys):
    key = (n_q, n_keys)
    if key not in _NC_CACHE:
        _NC_CACHE[key] = build_nc(n_q, n_keys)
    return _NC_CACHE[key]


def _pack(a, nt):
    """[nt*128, F] -> [128, nt, F] partition-major."""
    return np.ascontiguousarray(
        a.reshape(nt, 128, a.shape[1]).transpose(1, 0, 2))


def shard_inputs(x, context, Wq, Wk, Wv):
    """8 shards: (batch, query-half). Host-side layout prep only."""
    n_q = N // 2
    # weight folding: S = q k^T = x (Wk Wq^T applied to ctx)^T, so the
    # Q projection folds into the K-side weight (computed once, f32)
    wkq = (Wk.astype(np.float32) @ Wq.astype(np.float32).T)
    wk8 = _pack(wkq.astype(NP_F8), N_CT)
    wv8 = _pack(Wv.astype(NP_F8), N_CT)
    in_maps = []
    for core in range(NCORES):
        b, h = divmod(core, 2)
        xs = x[b, h * n_q:(h + 1) * n_q, :]
        xT = np.ascontiguousarray(xs.T)
        ctxT = np.ascontiguousarray(context[b].T)
        in_maps.append({
            "x16": _pack(xs.astype(NP_BF16), n_q // 128),
            "xT8": _pack(xT.astype(NP_F8), N_DT),
            "ctxT8": _pack(ctxT.astype(NP_F8), N_CT),
            "wk8": wk8, "wv8": wv8,
        })
    return in_maps


def unshard_output(results):
    n_q = N // 2
    out = np.empty((B, N, DIM), np.float32)
    for core in range(NCORES):
        b, h = divmod(core, 2)
        o = results[core]["out"]          # [128, n_qt, DIM] bf16
        out[b, h * n_q:(h + 1) * n_q, :] = (
            o.astype(np.float32).transpose(1, 0, 2).reshape(n_q, DIM))
    return out


def kernel(x, context, Wq, Wk, Wv):
    x = np.asarray(x, np.float32)
    context = np.asarray(context, np.float32)
    Wq = np.asarray(Wq, np.float32)
    Wk = np.asarray(Wk, np.float32)
    Wv = np.asarray(Wv, np.float32)
    nc = _get_nc(N // 2, M)
    in_maps = shard_inputs(x, context, Wq, Wk, Wv)
    res = run_bass_kernel_spmd(nc, in_maps, list(range(NCORES)))
    return unshard_output(res.results)

